# revision 20
# baseline (speedup 1.0000x reference)
"""Trainium2 Bass kernel for nn_CNNModel_42064909697048 (v3).

Per-image row/col stats (min/argmin/max/argmax/mean/median/argmedian over
both axes of each 28x28 image) -> 392 features -> 4-layer MLP -> softmax,
data-parallel over 8 NeuronCores.

v3 design:
- G=8 images per partition per super-tile (1024 images/ST, 16 STs/core).
- Stats computed on an fp16 slot-major copy of each image batch
  ([partition, slot s, 224 groups], slot rows contiguous) so every DVE
  tensor_tensor / scalar_tensor_tensor runs in the 2x packed perf mode.
  Means stay f32 (read the raw input). fp16 stats change the output by
  rel ~6.5e-3 on the actual data (threshold 2e-2).
- Median/min/max via a Batcher odd-even-mergesort network pruned to
  outputs {0,13,27} with per-comparator dead-output elision; min/copy ops
  skipped where the low output is dead.
- argmin/argmax/argmedian: eq-mask vs the extracted value, multiply by a
  static (1024 - slot) tile, per-group reduce-max => first-match index;
  the affine correction (idx = 1024 - raw) is folded into W1/b1 on host.
- MLP restructured to N=512 moving-dim matmuls over staged transposes.

Self-contained: hardcodes shapes/sharding; no sibling imports.
"""

import numpy as np

import concourse.bass as bass
import concourse.mybir as mybir
import concourse.tile as tile_mod
from concourse.tile import TileContext
from concourse.bass_utils import run_bass_kernel_spmd
from concourse.alu_op_type import AluOpType

# ---------------------------------------------------------------- constants
B_TOTAL = 131072
N_CORES = 8
B_CORE = B_TOTAL // N_CORES          # 16384
H = 28
D = 784
P = 128
G = 8                                # images per partition per super-tile
GH = G * H                           # 224 groups per axis view
FD = G * D                           # 6272 elems per partition per tile
N_ST = B_CORE // (P * G)             # 16 super-tiles
MED_IDX = 13
IDX_BASE = 1024.0                    # arg encoding: raw = 1024 - slot
F32 = mybir.dt.float32
F16 = mybir.dt.float16

# v3 feature order (f index 0..13); arg features last 6
FEATS = ["min_v1", "min_v2", "max_v1", "max_v2", "mean_1", "mean_2",
         "med_v1", "med_v2",
         "min_i1", "min_i2", "max_i1", "max_i2", "med_i1", "med_i2"]
REF_ORDER = ["min_v1", "min_i1", "min_v2", "min_i2",
             "max_v1", "max_i1", "max_v2", "max_i2",
             "mean_1", "mean_2",
             "med_v1", "med_i1", "med_v2", "med_i2"]
NFEAT = 392

# Batcher odd-even-mergesort on 28 lines pruned to outputs {0, 13, 27},
# with per-run liveness flags. Entry: (start, dist, step, count,
# need_min_out, need_max_out). After the network slot 0 = min,
# slot 13 = rank-13 (lower median), slot 27 = max.
# UNTOUCHED[r]: slot runs (start, step, count) not written by round r
# (r < 10), copied across the ping-pong buffers by the scalar engine.
UNTOUCHED = [[], [], [(24, 3, 2)], [(3, 1, 2), (11, 1, 2), (19, 1, 2), (23, 4, 2)], [(1, 5, 2), (7, 1, 3), (14, 1, 2), (17, 5, 2), (23, 1, 5)], [(0, 7, 2), (8, 7, 2), (16, 7, 2), (24, 1, 4)], [(0, 7, 2), (8, 7, 2), (16, 5, 2), (22, 1, 2)], [(0, 1, 4), (12, 1, 6), (19, 5, 2)], [(0, 1, 2), (14, 1, 3), (20, 6, 2), (27, 1, 1)], [(0, 15, 2), (16, 1, 3), (27, 1, 1)]]
NET_RUNS = [[(0, 1, 2, 14, True, True)], [(0, 2, 4, 7, True, True), (1, 2, 4, 7, True, True)], [(1, 1, 4, 7, True, True), (0, 4, 8, 3, True, True), (3, 4, 8, 3, True, True)], [(25, 1, 1, 1, True, True), (1, 4, 8, 3, True, True), (2, 4, 8, 3, True, True), (0, 8, 7, 2, True, True), (16, 8, 1, 1, True, True)], [(2, 2, 8, 3, True, True), (3, 2, 8, 3, True, True), (0, 16, 1, 1, True, True)], [(1, 1, 4, 6, True, True), (3, 1, 8, 3, True, True)], [(20, 4, 1, 1, True, True), (1, 8, 1, 6, True, True), (17, 8, 1, 3, True, True)], [(18, 2, 1, 1, True, True), (4, 4, 1, 4, True, True), (21, 4, 1, 3, True, True)], [(17, 1, 1, 1, True, True), (2, 2, 4, 3, True, True), (3, 2, 4, 3, True, True), (19, 2, 3, 2, True, True), (23, 2, 1, 1, True, True)], [(1, 1, 2, 7, True, True), (19, 1, 2, 4, True, True)], [(1, 16, 1, 6, False, True), (8, 16, 1, 3, True, False), (7, 16, 4, 2, True, True)], [(8, 8, 1, 3, False, True), (15, 8, 1, 1, False, True), (11, 8, 1, 4, True, False)], [(7, 4, 5, 2, False, True), (23, 4, 1, 1, False, True), (13, 4, 1, 2, True, False)], [(11, 2, 1, 1, False, True), (14, 2, 1, 1, True, False)], [(13, 1, 1, 1, True, False)]]

# ------------------------------------------------- tile tail-drain workaround
def _patched_drain_and_barrier(self, tick_clock, wait_clock):
    drain_inst = self.nc.sync.drain()
    wait_clock.add_sem_waits(
        drain_inst.ins, tile_mod.ScopedClock({None: tick_clock.global_clock})
    )
    si = drain_inst.ins.sync_info
    waits = list(si.on_wait or [])
    if len(waits) > 1:
        si.on_wait = waits[:1]
        for w in waits[1:]:
            d2 = self.nc.sync.drain()
            si2 = d2.ins.sync_info
            if si2 is None:
                d2.ins.sync_info = mybir.SyncInfo(on_wait=[w], on_update=[])
            else:
                si2.on_wait = [w]
    self.nc.all_engine_barrier()
    assert self.sems is not None
    popped = self.nc._tile_sem_poison_stack.pop()
    assert popped is self._sem_poison
    self.nc.clear_and_free_semaphores(list(self.sems.allocated().values()))
    self.nc.all_engine_barrier()


tile_mod.TileContext._drain_and_barrier = _patched_drain_and_barrier


# ------------------------------------------------------------- bass program
def build_nc(n_st: int = N_ST, debug_features: bool = False):
    nc = bass.Bass()
    rows = P * G * n_st
    t_in = nc.dram_tensor("t", [rows, D], F32, kind="ExternalInput")
    w1 = nc.dram_tensor("w1", [NFEAT, 270], F32, kind="ExternalInput")
    b1 = nc.dram_tensor("b1", [270, 1], F32, kind="ExternalInput")
    w2 = nc.dram_tensor("w2", [270, 90], F32, kind="ExternalInput")
    b2 = nc.dram_tensor("b2", [90, 1], F32, kind="ExternalInput")
    w3 = nc.dram_tensor("w3", [90, 30], F32, kind="ExternalInput")
    b3 = nc.dram_tensor("b3", [30, 1], F32, kind="ExternalInput")
    w4 = nc.dram_tensor("w4", [30, 10], F32, kind="ExternalInput")
    b4 = nc.dram_tensor("b4", [10, 1], F32, kind="ExternalInput")
    idn = nc.dram_tensor("idn", [P, P], F32, kind="ExternalInput")
    iot = nc.dram_tensor("iot", [P, H * GH], F16, kind="ExternalInput")
    if debug_features:
        y_out = nc.dram_tensor("y", [P * n_st, 14 * GH], F32,
                               kind="ExternalOutput")
    else:
        y_out = nc.dram_tensor("y", [rows, 10], F32, kind="ExternalOutput")

    RMAX = AluOpType.max
    RADD = AluOpType.add
    AXX = mybir.AxisListType.X
    RELU = mybir.ActivationFunctionType.Relu
    EXP = mybir.ActivationFunctionType.Exp

    K1 = [(0, 112), (112, 112), (224, 112), (336, 56)]
    M1 = [(0, 128), (128, 128), (256, 14)]
    K2 = [(0, 128), (128, 128), (256, 14)]
    TCH = [(0, 4), (4, 4), (8, 4), (12, 2)]

    with TileContext(nc) as tc:
        with (
            tc.tile_pool(name="wpool", bufs=1) as wpool,
            tc.tile_pool(name="xpool", bufs=1) as xpool,
            tc.tile_pool(name="smpool", bufs=2) as smpool,
            tc.tile_pool(name="vpool", bufs=1) as vpool,
            tc.tile_pool(name="opool", bufs=1) as opool,
            tc.tile_pool(name="fpool", bufs=2) as fpool,
            tc.tile_pool(name="ftpool", bufs=1) as ftpool,
            tc.tile_pool(name="fgpool", bufs=2) as fgpool,
            tc.tile_pool(name="mpool", bufs=1) as mpool,
            tc.tile_pool(name="ypool", bufs=2) as ypool,
            tc.tile_pool(name="psT", bufs=2, space="PSUM") as psT,
            tc.tile_pool(name="ps1", bufs=2, space="PSUM") as ps1p,
            tc.tile_pool(name="psB", bufs=2, space="PSUM") as psBp,
            tc.tile_pool(name="psS", bufs=2, space="PSUM") as psSp,
        ):
            # ---- static weights into SBUF
            w1_t = []
            for ci, (k0, kc) in enumerate(K1):
                wt = wpool.tile([kc, 270], F32, name=f"w1_{ci}", tag=f"w1_{ci}")
                nc.sync.dma_start(wt[:], w1[k0:k0 + kc, :])
                w1_t.append(wt)
            b1_t = []
            for mi, (m0, mc) in enumerate(M1):
                bt = wpool.tile([mc, 1], F32, name=f"b1_{mi}", tag=f"b1_{mi}")
                nc.sync.dma_start(bt[:], b1[m0:m0 + mc, :])
                b1_t.append(bt)
            w2_t = []
            for ci, (k0, kc) in enumerate(K2):
                wt = wpool.tile([kc, 90], F32, name=f"w2_{ci}", tag=f"w2_{ci}")
                nc.sync.dma_start(wt[:], w2[k0:k0 + kc, :])
                w2_t.append(wt)
            b2_t = wpool.tile([90, 1], F32, name="b2", tag="b2")
            nc.sync.dma_start(b2_t[:], b2[:, :])
            w3_t = wpool.tile([90, 30], F32, name="w3", tag="w3")
            nc.sync.dma_start(w3_t[:], w3[:, :])
            b3_t = wpool.tile([30, 1], F32, name="b3", tag="b3")
            nc.sync.dma_start(b3_t[:], b3[:, :])
            w4_t = wpool.tile([30, 10], F32, name="w4", tag="w4")
            nc.sync.dma_start(w4_t[:], w4[:, :])
            b4_t = wpool.tile([10, 1], F32, name="b4", tag="b4")
            nc.sync.dma_start(b4_t[:], b4[:, :])
            idn_t = wpool.tile([P, P], F32, name="idn", tag="idn")
            nc.sync.dma_start(idn_t[:], idn[:, :])
            iot_t = wpool.tile([P, H * GH], F16, name="iot", tag="iot")
            nc.sync.dma_start(iot_t[:], iot[:, :])

            for it in range(n_st):
                X = xpool.tile([P, FD], F32, name="x", tag="x")
                nc.sync.dma_start(
                    X.rearrange("p (g d) -> p g d", d=D),
                    t_in[it * P * G:(it + 1) * P * G, :]
                        .rearrange("(g p) d -> p g d", p=P))

                # dual-axis slot-major fp16 copy XSM[p, a, s, x]:
                #   a=0 (axis1): XSM[p,0,r,(g c)] ; a=1 (axis2): XSM[p,1,c,(g r)]
                XSM = smpool.tile([P, 2 * H * GH], F16, name="xsm", tag="xsm")
                XSM4 = XSM.rearrange("p (a s x) -> p a s x", s=H, x=GH)
                nc.scalar.copy(
                    XSM[:, 0:H * GH].rearrange("p (r g c) -> p r g c",
                                               g=G, c=H),
                    X.rearrange("p (g r c) -> p r g c", r=H, c=H))
                nc.scalar.copy(
                    XSM[:, H * GH:].rearrange("p (s x) -> p s x", x=GH),
                    X.rearrange("p (x c) -> p c x", c=H))

                F2 = fpool.tile([P, 14 * GH], F32, name="feat", tag="feat")
                TGT = fpool.tile([P, 8 * GH], F16, name="tgt", tag="tgt")

                def blk2(f):
                    return F2[:, f * GH:(f + 2) * GH] \
                        .rearrange("p (a x) -> p a x", x=GH)

                # ---- means: fp16 fold-tree over slots (sum), staged in O
                # (free until sort round 0 overwrites it), both axes at once
                O = opool.tile([P, 2 * H * GH], F16, name="osel", tag="osel")
                O4 = O.rearrange("p (a s x) -> p a s x", s=H, x=GH)
                nc.vector.tensor_tensor(O4[:, :, 0:14, :],
                                        XSM4[:, :, 0:14, :],
                                        XSM4[:, :, 14:28, :], op=RADD)
                for (lo_, hi_, k_) in ((0, 7, 7), (0, 4, 3), (0, 2, 2),
                                       (0, 1, 1)):
                    nc.vector.tensor_tensor(O4[:, :, lo_:lo_ + k_, :],
                                            O4[:, :, lo_:lo_ + k_, :],
                                            O4[:, :, hi_:hi_ + k_, :],
                                            op=RADD)
                nc.scalar.copy(blk2(4), O4[:, :, 0, :])

                # ---- dual-axis sort on fp16 slot-major scratch.
                # Rounds 0-9 ping-pong XSM -> O -> V -> O -> ... (min/max write
                # the other buffer: no temp, no copy-back; scalar copies only
                # the untouched slots, concurrently). Rounds 10+ run in-place
                # on V with liveness flags.
                V = vpool.tile([P, 2 * H * GH], F16, name="vsort", tag="vsort")
                V4 = V.rearrange("p (a s x) -> p a s x", s=H, x=GH)

                def _sl(st_, step, n):
                    return slice(st_, st_ + step * (n - 1) + 1, step) \
                        if n > 1 else slice(st_, st_ + 1)

                for r, runs in enumerate(NET_RUNS):
                    if r < 10:
                        bin_ = XSM4 if r == 0 else (O4 if r % 2 == 1 else V4)
                        bout = O4 if r % 2 == 0 else V4
                        for (st_, d, step, n, nm, nx) in runs:
                            sl = _sl(st_, step, n)
                            sh = _sl(st_ + d, step, n)
                            nc.vector.tensor_tensor(
                                bout[:, :, sl, :], bin_[:, :, sl, :],
                                bin_[:, :, sh, :], op=AluOpType.min)
                            nc.vector.tensor_tensor(
                                bout[:, :, sh, :], bin_[:, :, sl, :],
                                bin_[:, :, sh, :], op=RMAX)
                        for (u0, ustep, ucnt) in UNTOUCHED[r]:
                            us = _sl(u0, ustep, ucnt)
                            nc.scalar.copy(bout[:, :, us, :],
                                           bin_[:, :, us, :])
                    else:
                        for (st_, d, step, n, nm, nx) in runs:
                            sl = _sl(st_, step, n)
                            sh = _sl(st_ + d, step, n)
                            lo = V4[:, :, sl, :]
                            hi = V4[:, :, sh, :]
                            if nm and nx:
                                tt = O4[:, :, 0:n, :]
                                nc.vector.tensor_tensor(tt, lo, hi,
                                                        op=AluOpType.min)
                                nc.vector.tensor_tensor(hi, lo, hi, op=RMAX)
                                nc.scalar.copy(lo, tt)
                            elif nm:
                                nc.vector.tensor_tensor(lo, lo, hi,
                                                        op=AluOpType.min)
                            else:
                                nc.vector.tensor_tensor(hi, lo, hi, op=RMAX)
                # extract value features (axis pairs adjacent in F2/TGT):
                # slot0 -> (min_v1,min_v2)=f0,1; slot13 -> f6,7; slot27 -> f2,3
                for s_, f_ in ((0, 0), (27, 2), (MED_IDX, 6)):
                    nc.scalar.copy(blk2(f_), V4[:, :, s_, :])
                    nc.scalar.copy(
                        TGT[:, f_ * GH:(f_ + 2) * GH]
                            .rearrange("p (a x) -> p a x", x=GH),
                        V4[:, :, s_, :])

                # ---- arg features, axis-paired: eq-mask * (1024 - slot),
                # fp16 fold-tree max => first-match index (O reused as scratch)
                iot_bc = iot_t.rearrange("p (u s x) -> p u s x", u=1, x=GH) \
                              .broadcast_to([P, 2, H, GH])
                for (fa, fv) in ((8, 0), (10, 2), (12, 6)):
                    bc = TGT[:, fv * GH:(fv + 2) * GH] \
                        .rearrange("p (a u x) -> p a u x", u=1, x=GH) \
                        .broadcast_to([P, 2, H, GH])
                    nc.vector.tensor_tensor(O4, XSM4, bc,
                                            op=AluOpType.is_equal)
                    nc.vector.tensor_tensor(O4, O4, iot_bc,
                                            op=AluOpType.mult)
                    nc.vector.tensor_tensor(O4[:, :, 0:14, :],
                                            O4[:, :, 0:14, :],
                                            O4[:, :, 14:28, :], op=RMAX)
                    for (lo_, hi_, k_) in ((0, 7, 7), (0, 4, 3), (0, 2, 2),
                                           (0, 1, 1)):
                        nc.vector.tensor_tensor(O4[:, :, lo_:lo_ + k_, :],
                                                O4[:, :, lo_:lo_ + k_, :],
                                                O4[:, :, hi_:hi_ + k_, :],
                                                op=RMAX)
                    nc.scalar.copy(blk2(fa), O4[:, :, 0, :])

                if debug_features:
                    nc.sync.dma_start(y_out[P * it:P * (it + 1), :], F2[:])
                    continue

                # ---- MLP on two N=512 halves (groups 4h..4h+3)
                F2f = F2.rearrange("p (f x) -> p f x", x=GH)
                Y = ypool.tile([P, G * 10], F32, name="yt", tag="yt")
                for hh in (0, 1):
                    fT = [ftpool.tile([nf * H, 512], F32, name=f"ft_{ci}",
                                      tag=f"ft_{ci}")
                          for ci, (f0, nf) in enumerate(TCH)]
                    for gg in range(4):
                        g = 4 * hh + gg
                        fg = fgpool.tile([P, NFEAT], F32, name=f"fg_{gg}",
                                         tag="fg")
                        nc.scalar.copy(
                            fg.rearrange("p (f c) -> p f c", c=H),
                            F2f[:, :, g * H:(g + 1) * H])
                        for ci, (f0, nf) in enumerate(TCH):
                            kc = nf * H
                            pt = psT.tile([kc, P], F32, name=f"pt_{ci}_{gg}",
                                          tag="pt")
                            nc.tensor.transpose(
                                pt[:], fg[:, f0 * H:f0 * H + kc], idn_t[:])
                            nc.scalar.copy(fT[ci][:, gg * P:(gg + 1) * P],
                                           pt[:])

                    a1 = []
                    for mi, (m0, mc) in enumerate(M1):
                        ps = ps1p.tile([mc, 512], F32, name=f"l1_{mi}",
                                       tag="l1")
                        for ci, (k0, kc) in enumerate(K1):
                            nc.tensor.matmul(ps[:],
                                             w1_t[ci][:, m0:m0 + mc],
                                             fT[ci][:],
                                             start=(ci == 0), stop=(ci == 3))
                        am = mpool.tile([mc, 512], F32, name=f"a1_{mi}",
                                        tag=f"a1_{mi}")
                        nc.scalar.activation(am[:], ps[:], RELU,
                                             bias=b1_t[mi][:], scale=1.0)
                        a1.append(am)

                    ps2 = psBp.tile([90, 512], F32, name="l2", tag="lB")
                    for ci, (k0, kc) in enumerate(K2):
                        nc.tensor.matmul(ps2[:], w2_t[ci][0:kc, :],
                                         a1[ci][0:kc, :],
                                         start=(ci == 0), stop=(ci == 2))
                    a2 = mpool.tile([90, 512], F32, name="a2", tag="a2")
                    nc.scalar.activation(a2[:], ps2[:], RELU,
                                         bias=b2_t[:], scale=1.0)

                    ps3 = psBp.tile([30, 512], F32, name="l3", tag="lB")
                    nc.tensor.matmul(ps3[:], w3_t[:], a2[:],
                                     start=True, stop=True)
                    a3 = mpool.tile([30, 512], F32, name="a3", tag="a3")
                    nc.scalar.activation(a3[:], ps3[:], RELU,
                                         bias=b3_t[:], scale=1.0)

                    ps4 = psBp.tile([10, 512], F32, name="l4", tag="lB")
                    nc.tensor.matmul(ps4[:], w4_t[:], a3[:],
                                     start=True, stop=True)
                    ex = mpool.tile([10, 512], F32, name="ex", tag="ex")
                    nc.scalar.activation(ex[:], ps4[:], EXP,
                                         bias=b4_t[:], scale=1.0)

                    for gg in range(4):
                        g = 4 * hh + gg
                        pst = psSp.tile([P, 16], F32, name=f"sm_{gg}",
                                        tag="sm")
                        nc.tensor.transpose(pst[:, 0:10],
                                            ex[:, gg * P:(gg + 1) * P],
                                            idn_t[0:10, 0:10])
                        sumv = mpool.tile([P, 1], F32, name=f"sv_{gg}",
                                          tag="sv")
                        nc.vector.tensor_reduce(sumv[:], pst[:, 0:10],
                                                axis=AXX, op=RADD)
                        rcp = mpool.tile([P, 1], F32, name=f"rc_{gg}",
                                         tag="rc")
                        nc.vector.reciprocal(rcp[:], sumv[:])
                        nc.vector.tensor_scalar_mul(
                            Y[:, g * 10:(g + 1) * 10], pst[:, 0:10], rcp[:])

                nc.sync.dma_start(
                    y_out[it * P * G:(it + 1) * P * G, :]
                        .rearrange("(g p) c -> p g c", p=P),
                    Y.rearrange("p (g c) -> p g c", c=10))

    _split_excess_waits(nc)
    return nc


MAX_WAITS = 1


def _split_excess_waits(nc):
    """Walrus in this container rejects instructions with >MAX_WAITS sem
    waits; hoist the excess onto NoOp carriers inserted just before."""
    import bass_rust
    ctr = [0]
    for f in nc.m.functions:
        for blkk in f.blocks:
            insts = list(blkk.instructions)
            out = []
            changed = False
            for inst in insts:
                si = inst.sync_info
                waits = list(si.on_wait) if (si and si.on_wait) else []
                if len(waits) > MAX_WAITS:
                    changed = True
                    excess = waits[:-MAX_WAITS]
                    si.on_wait = waits[-MAX_WAITS:]
                    for k in range(0, len(excess), MAX_WAITS):
                        nop = bass_rust.InstNoOp(
                            name=f"WSPLIT-{ctr[0]}", ins=[], outs=[])
                        ctr[0] += 1
                        nop.engine = inst.engine
                        nop.sync_info = mybir.SyncInfo(
                            on_wait=excess[k:k + MAX_WAITS], on_update=[])
                        out.append(nop)
                out.append(inst)
            if changed:
                blkk.instructions = out


# ------------------------------------------------------------- numpy driver
def _prep_weights(W1, b1, W2, b2, W3, b3, W4, b4):
    """Fold per-feature affine corrections into W1/b1 (v3 feature order);
    return device weight dict."""
    scale = np.ones(NFEAT, np.float64)
    offset = np.zeros(NFEAT, np.float64)
    ref_off = {n: i * H for i, n in enumerate(REF_ORDER)}
    perm = np.zeros(NFEAT, np.int64)
    for f, name in enumerate(FEATS):
        cols = slice(f * H, (f + 1) * H)
        perm[cols] = ref_off[name] + np.arange(H)
        if name.startswith("mean"):
            scale[cols] = 1.0 / H
        elif name[4] == "i":
            # raw = 1024 - slot  =>  idx = 1024 - raw
            scale[cols] = -1.0
            offset[cols] = IDX_BASE
    W1p = W1.astype(np.float64)[:, perm]
    W1_eff = W1p * scale[None, :]
    b1_eff = b1.astype(np.float64) + W1p @ offset
    iot_row = np.repeat(IDX_BASE - np.arange(H, dtype=np.float32), GH)
    return {
        "w1": np.ascontiguousarray(W1_eff.T.astype(np.float32)),
        "b1": b1_eff.astype(np.float32).reshape(-1, 1),
        "w2": np.ascontiguousarray(W2.T.astype(np.float32)),
        "b2": b2.reshape(-1, 1).astype(np.float32),
        "w3": np.ascontiguousarray(W3.T.astype(np.float32)),
        "b3": b3.reshape(-1, 1).astype(np.float32),
        "w4": np.ascontiguousarray(W4.T.astype(np.float32)),
        "b4": b4.reshape(-1, 1).astype(np.float32),
        "idn": np.eye(P, dtype=np.float32),
        "iot": np.broadcast_to(iot_row[None, :].astype(np.float16),
                               (P, H * GH)).copy(),
    }


_NC_CACHE = {}


def _get_nc(n_st, debug_features=False):
    key = (n_st, debug_features)
    if key not in _NC_CACHE:
        _NC_CACHE[key] = build_nc(n_st, debug_features)
    return _NC_CACHE[key]


def run(t, weights, n_st=N_ST, debug_features=False, trace=False):
    nc = _get_nc(n_st, debug_features)
    rows = P * G * n_st
    in_maps = []
    for c in range(N_CORES):
        m = {"t": np.ascontiguousarray(t[c * B_CORE:c * B_CORE + rows])}
        m.update(weights)
        in_maps.append(m)
    res = run_bass_kernel_spmd(nc, in_maps, core_ids=list(range(N_CORES)),
                               trace=trace)
    outs = [r["y"] for r in res.results]
    return outs, res


def kernel(t, W1, b1, W2, b2, W3, b3, W4, b4):
    weights = _prep_weights(W1, b1, W2, b2, W3, b3, W4, b4)
    outs, _ = run(t, weights)
    y = np.concatenate(outs, axis=0)
    return np.ascontiguousarray(y.astype(np.float32))


# revision 21
# speedup vs baseline: 1.1950x; 1.1950x over previous
"""Trainium2 Bass kernel for nn_CNNModel_42064909697048 (v3).

Per-image row/col stats (min/argmin/max/argmax/mean/median/argmedian over
both axes of each 28x28 image) -> 392 features -> 4-layer MLP -> softmax,
data-parallel over 8 NeuronCores.

v3 design:
- G=8 images per partition per super-tile (1024 images/ST, 16 STs/core).
- Stats computed on an fp16 slot-major copy of each image batch
  ([partition, slot s, 224 groups], slot rows contiguous) so every DVE
  tensor_tensor / scalar_tensor_tensor runs in the 2x packed perf mode.
  Means stay f32 (read the raw input). fp16 stats change the output by
  rel ~6.5e-3 on the actual data (threshold 2e-2).
- Median/min/max via a Batcher odd-even-mergesort network pruned to
  outputs {0,13,27} with per-comparator dead-output elision; min/copy ops
  skipped where the low output is dead.
- argmin/argmax/argmedian: eq-mask vs the extracted value, multiply by a
  static (1024 - slot) tile, per-group reduce-max => first-match index;
  the affine correction (idx = 1024 - raw) is folded into W1/b1 on host.
- MLP restructured to N=512 moving-dim matmuls over staged transposes.

Self-contained: hardcodes shapes/sharding; no sibling imports.
"""

import numpy as np

import concourse.bass as bass
import concourse.mybir as mybir
import concourse.tile as tile_mod
from concourse.tile import TileContext
from concourse.bass_utils import run_bass_kernel_spmd
from concourse.alu_op_type import AluOpType

# ---------------------------------------------------------------- constants
B_TOTAL = 131072
N_CORES = 8
B_CORE = B_TOTAL // N_CORES          # 16384
H = 28
D = 784
P = 128
G = 8                                # images per partition per super-tile
GH = G * H                           # 224 groups per axis view
FD = G * D                           # 6272 elems per partition per tile
N_ST = B_CORE // (P * G)             # 16 super-tiles
MED_IDX = 13
IDX_BASE = 1024.0                    # arg encoding: raw = 1024 - slot
F32 = mybir.dt.float32
F16 = mybir.dt.float16

# v3 feature order (f index 0..13); arg features last 6
FEATS = ["min_v1", "min_v2", "max_v1", "max_v2", "mean_1", "mean_2",
         "med_v1", "med_v2",
         "min_i1", "min_i2", "max_i1", "max_i2", "med_i1", "med_i2"]
REF_ORDER = ["min_v1", "min_i1", "min_v2", "min_i2",
             "max_v1", "max_i1", "max_v2", "max_i2",
             "mean_1", "mean_2",
             "med_v1", "med_i1", "med_v2", "med_i2"]
NFEAT = 392

# Batcher odd-even-mergesort on 28 lines pruned to outputs {0, 13, 27},
# with per-run liveness flags. Entry: (start, dist, step, count,
# need_min_out, need_max_out). After the network slot 0 = min,
# slot 13 = rank-13 (lower median), slot 27 = max.
# UNTOUCHED[r]: slot runs (start, step, count) not written by round r
# (r < 10), copied across the ping-pong buffers by the scalar engine.
UNTOUCHED = [[], [], [(24, 3, 2)], [(3, 1, 2), (11, 1, 2), (19, 1, 2), (23, 4, 2)], [(1, 5, 2), (7, 1, 3), (14, 1, 2), (17, 5, 2), (23, 1, 5)], [(0, 7, 2), (8, 7, 2), (16, 7, 2), (24, 1, 4)], [(0, 7, 2), (8, 7, 2), (16, 5, 2), (22, 1, 2)], [(0, 1, 4), (12, 1, 6), (19, 5, 2)], [(0, 1, 2), (14, 1, 3), (20, 6, 2), (27, 1, 1)], [(0, 15, 2), (16, 1, 3), (27, 1, 1)]]
NET_RUNS = [[(0, 1, 2, 14, True, True)], [(0, 2, 4, 7, True, True), (1, 2, 4, 7, True, True)], [(1, 1, 4, 7, True, True), (0, 4, 8, 3, True, True), (3, 4, 8, 3, True, True)], [(25, 1, 1, 1, True, True), (1, 4, 8, 3, True, True), (2, 4, 8, 3, True, True), (0, 8, 7, 2, True, True), (16, 8, 1, 1, True, True)], [(2, 2, 8, 3, True, True), (3, 2, 8, 3, True, True), (0, 16, 1, 1, True, True)], [(1, 1, 4, 6, True, True), (3, 1, 8, 3, True, True)], [(20, 4, 1, 1, True, True), (1, 8, 1, 6, True, True), (17, 8, 1, 3, True, True)], [(18, 2, 1, 1, True, True), (4, 4, 1, 4, True, True), (21, 4, 1, 3, True, True)], [(17, 1, 1, 1, True, True), (2, 2, 4, 3, True, True), (3, 2, 4, 3, True, True), (19, 2, 3, 2, True, True), (23, 2, 1, 1, True, True)], [(1, 1, 2, 7, True, True), (19, 1, 2, 4, True, True)], [(1, 16, 1, 6, False, True), (8, 16, 1, 3, True, False), (7, 16, 4, 2, True, True)], [(8, 8, 1, 3, False, True), (15, 8, 1, 1, False, True), (11, 8, 1, 4, True, False)], [(7, 4, 5, 2, False, True), (23, 4, 1, 1, False, True), (13, 4, 1, 2, True, False)], [(11, 2, 1, 1, False, True), (14, 2, 1, 1, True, False)], [(13, 1, 1, 1, True, False)]]

# ------------------------------------------------- tile tail-drain workaround
def _patched_drain_and_barrier(self, tick_clock, wait_clock):
    drain_inst = self.nc.sync.drain()
    wait_clock.add_sem_waits(
        drain_inst.ins, tile_mod.ScopedClock({None: tick_clock.global_clock})
    )
    si = drain_inst.ins.sync_info
    waits = list(si.on_wait or [])
    if len(waits) > 1:
        si.on_wait = waits[:1]
        for w in waits[1:]:
            d2 = self.nc.sync.drain()
            si2 = d2.ins.sync_info
            if si2 is None:
                d2.ins.sync_info = mybir.SyncInfo(on_wait=[w], on_update=[])
            else:
                si2.on_wait = [w]
    self.nc.all_engine_barrier()
    assert self.sems is not None
    popped = self.nc._tile_sem_poison_stack.pop()
    assert popped is self._sem_poison
    self.nc.clear_and_free_semaphores(list(self.sems.allocated().values()))
    self.nc.all_engine_barrier()


tile_mod.TileContext._drain_and_barrier = _patched_drain_and_barrier


# ------------------------------------------------------------- bass program
def build_nc(n_st: int = N_ST, debug_features: bool = False):
    nc = bass.Bass()
    rows = P * G * n_st
    t_in = nc.dram_tensor("t", [rows, D], F32, kind="ExternalInput")
    w1 = nc.dram_tensor("w1", [NFEAT, 270], F32, kind="ExternalInput")
    b1 = nc.dram_tensor("b1", [270, 1], F32, kind="ExternalInput")
    w2 = nc.dram_tensor("w2", [270, 90], F32, kind="ExternalInput")
    b2 = nc.dram_tensor("b2", [90, 1], F32, kind="ExternalInput")
    w3 = nc.dram_tensor("w3", [90, 30], F32, kind="ExternalInput")
    b3 = nc.dram_tensor("b3", [30, 1], F32, kind="ExternalInput")
    w4 = nc.dram_tensor("w4", [30, 10], F32, kind="ExternalInput")
    b4 = nc.dram_tensor("b4", [10, 1], F32, kind="ExternalInput")
    idn = nc.dram_tensor("idn", [P, P], F32, kind="ExternalInput")
    iot = nc.dram_tensor("iot", [P, H * GH], F16, kind="ExternalInput")
    if debug_features:
        y_out = nc.dram_tensor("y", [P * n_st, 14 * GH], F32,
                               kind="ExternalOutput")
    else:
        y_out = nc.dram_tensor("y", [rows, 10], F32, kind="ExternalOutput")

    RMAX = AluOpType.max
    RADD = AluOpType.add
    AXX = mybir.AxisListType.X
    RELU = mybir.ActivationFunctionType.Relu
    EXP = mybir.ActivationFunctionType.Exp

    K1 = [(0, 112), (112, 112), (224, 112), (336, 56)]
    M1 = [(0, 128), (128, 128), (256, 14)]
    K2 = [(0, 128), (128, 128), (256, 14)]
    TCH = [(0, 4), (4, 4), (8, 4), (12, 2)]

    with TileContext(nc) as tc:
        with (
            tc.tile_pool(name="wpool", bufs=1) as wpool,
            tc.tile_pool(name="xpool", bufs=2) as xpool,
            tc.tile_pool(name="smpool", bufs=1) as smpool,
            tc.tile_pool(name="vpool", bufs=1) as vpool,
            tc.tile_pool(name="opool", bufs=1) as opool,
            tc.tile_pool(name="fpool", bufs=2) as fpool,
            tc.tile_pool(name="ftpool", bufs=1) as ftpool,
            tc.tile_pool(name="fgpool", bufs=2) as fgpool,
            tc.tile_pool(name="mpool", bufs=1) as mpool,
            tc.tile_pool(name="ypool", bufs=2) as ypool,
            tc.tile_pool(name="psT", bufs=2, space="PSUM") as psT,
            tc.tile_pool(name="ps1", bufs=2, space="PSUM") as ps1p,
            tc.tile_pool(name="psB", bufs=2, space="PSUM") as psBp,
            tc.tile_pool(name="psS", bufs=2, space="PSUM") as psSp,
        ):
            # ---- static weights into SBUF
            w1_t = []
            for ci, (k0, kc) in enumerate(K1):
                wt = wpool.tile([kc, 270], F32, name=f"w1_{ci}", tag=f"w1_{ci}")
                nc.sync.dma_start(wt[:], w1[k0:k0 + kc, :])
                w1_t.append(wt)
            b1_t = []
            for mi, (m0, mc) in enumerate(M1):
                bt = wpool.tile([mc, 1], F32, name=f"b1_{mi}", tag=f"b1_{mi}")
                nc.sync.dma_start(bt[:], b1[m0:m0 + mc, :])
                b1_t.append(bt)
            w2_t = []
            for ci, (k0, kc) in enumerate(K2):
                wt = wpool.tile([kc, 90], F32, name=f"w2_{ci}", tag=f"w2_{ci}")
                nc.sync.dma_start(wt[:], w2[k0:k0 + kc, :])
                w2_t.append(wt)
            b2_t = wpool.tile([90, 1], F32, name="b2", tag="b2")
            nc.sync.dma_start(b2_t[:], b2[:, :])
            w3_t = wpool.tile([90, 30], F32, name="w3", tag="w3")
            nc.sync.dma_start(w3_t[:], w3[:, :])
            b3_t = wpool.tile([30, 1], F32, name="b3", tag="b3")
            nc.sync.dma_start(b3_t[:], b3[:, :])
            w4_t = wpool.tile([30, 10], F32, name="w4", tag="w4")
            nc.sync.dma_start(w4_t[:], w4[:, :])
            b4_t = wpool.tile([10, 1], F32, name="b4", tag="b4")
            nc.sync.dma_start(b4_t[:], b4[:, :])
            idn_t = wpool.tile([P, P], F32, name="idn", tag="idn")
            nc.sync.dma_start(idn_t[:], idn[:, :])
            iot_t = wpool.tile([P, H * GH], F16, name="iot", tag="iot")
            nc.sync.dma_start(iot_t[:], iot[:, :])

            for it in range(n_st):
                X = xpool.tile([P, FD], F32, name="x", tag="x")
                nc.sync.dma_start(
                    X.rearrange("p (g d) -> p g d", d=D),
                    t_in[it * P * G:(it + 1) * P * G, :]
                        .rearrange("(g p) d -> p g d", p=P))

                # dual-axis slot-major fp16 copy XSM[p, a, s, x]:
                #   a=0 (axis1): XSM[p,0,r,(g c)] ; a=1 (axis2): XSM[p,1,c,(g r)]
                XSM = smpool.tile([P, 2 * H * GH], F16, name="xsm", tag="xsm")
                XSM4 = XSM.rearrange("p (a s x) -> p a s x", s=H, x=GH)
                nc.scalar.copy(
                    XSM[:, 0:H * GH].rearrange("p (r g c) -> p r g c",
                                               g=G, c=H),
                    X.rearrange("p (g r c) -> p r g c", r=H, c=H))
                nc.scalar.copy(
                    XSM[:, H * GH:].rearrange("p (s x) -> p s x", x=GH),
                    X.rearrange("p (x c) -> p c x", c=H))

                F2 = fpool.tile([P, 14 * GH], F32, name="feat", tag="feat")
                TGT = fpool.tile([P, 8 * GH], F16, name="tgt", tag="tgt")

                def blk2(f):
                    return F2[:, f * GH:(f + 2) * GH] \
                        .rearrange("p (a x) -> p a x", x=GH)

                # ---- means: fp16 fold-tree over slots (sum), staged in O
                # (free until sort round 0 overwrites it), both axes at once
                O = opool.tile([P, 2 * H * GH], F16, name="osel", tag="osel")
                O4 = O.rearrange("p (a s x) -> p a s x", s=H, x=GH)
                nc.vector.tensor_tensor(O4[:, :, 0:14, :],
                                        XSM4[:, :, 0:14, :],
                                        XSM4[:, :, 14:28, :], op=RADD)
                for (lo_, hi_, k_) in ((0, 7, 7), (0, 4, 3), (0, 2, 2),
                                       (0, 1, 1)):
                    nc.vector.tensor_tensor(O4[:, :, lo_:lo_ + k_, :],
                                            O4[:, :, lo_:lo_ + k_, :],
                                            O4[:, :, hi_:hi_ + k_, :],
                                            op=RADD)
                nc.scalar.copy(blk2(4), O4[:, :, 0, :])

                # ---- dual-axis sort on fp16 slot-major scratch.
                # Rounds 0-9 ping-pong XSM -> O -> V -> O -> ... (min/max write
                # the other buffer: no temp, no copy-back; scalar copies only
                # the untouched slots, concurrently). Rounds 10+ run in-place
                # on V with liveness flags.
                V = vpool.tile([P, 2 * H * GH], F16, name="vsort", tag="vsort")
                V4 = V.rearrange("p (a s x) -> p a s x", s=H, x=GH)

                def _sl(st_, step, n):
                    return slice(st_, st_ + step * (n - 1) + 1, step) \
                        if n > 1 else slice(st_, st_ + 1)

                for r, runs in enumerate(NET_RUNS):
                    if r < 10:
                        bin_ = XSM4 if r == 0 else (O4 if r % 2 == 1 else V4)
                        bout = O4 if r % 2 == 0 else V4
                        for (st_, d, step, n, nm, nx) in runs:
                            sl = _sl(st_, step, n)
                            sh = _sl(st_ + d, step, n)
                            nc.vector.tensor_tensor(
                                bout[:, :, sl, :], bin_[:, :, sl, :],
                                bin_[:, :, sh, :], op=AluOpType.min)
                            nc.vector.tensor_tensor(
                                bout[:, :, sh, :], bin_[:, :, sl, :],
                                bin_[:, :, sh, :], op=RMAX)
                        for (u0, ustep, ucnt) in UNTOUCHED[r]:
                            us = _sl(u0, ustep, ucnt)
                            nc.scalar.copy(bout[:, :, us, :],
                                           bin_[:, :, us, :])
                    else:
                        for (st_, d, step, n, nm, nx) in runs:
                            sl = _sl(st_, step, n)
                            sh = _sl(st_ + d, step, n)
                            lo = V4[:, :, sl, :]
                            hi = V4[:, :, sh, :]
                            if nm and nx:
                                tt = O4[:, :, 0:n, :]
                                nc.vector.tensor_tensor(tt, lo, hi,
                                                        op=AluOpType.min)
                                nc.vector.tensor_tensor(hi, lo, hi, op=RMAX)
                                nc.scalar.copy(lo, tt)
                            elif nm:
                                nc.vector.tensor_tensor(lo, lo, hi,
                                                        op=AluOpType.min)
                            else:
                                nc.vector.tensor_tensor(hi, lo, hi, op=RMAX)
                # extract value features (axis pairs adjacent in F2/TGT):
                # slot0 -> (min_v1,min_v2)=f0,1; slot13 -> f6,7; slot27 -> f2,3
                for s_, f_ in ((0, 0), (27, 2), (MED_IDX, 6)):
                    nc.scalar.copy(blk2(f_), V4[:, :, s_, :])
                    nc.scalar.copy(
                        TGT[:, f_ * GH:(f_ + 2) * GH]
                            .rearrange("p (a x) -> p a x", x=GH),
                        V4[:, :, s_, :])

                # ---- arg features, axis-paired: eq-mask * (1024 - slot),
                # fp16 fold-tree max => first-match index (O reused as scratch)
                iot_bc = iot_t.rearrange("p (u s x) -> p u s x", u=1, x=GH) \
                              .broadcast_to([P, 2, H, GH])
                for (fa, fv) in ((8, 0), (10, 2), (12, 6)):
                    bc = TGT[:, fv * GH:(fv + 2) * GH] \
                        .rearrange("p (a u x) -> p a u x", u=1, x=GH) \
                        .broadcast_to([P, 2, H, GH])
                    nc.vector.tensor_tensor(O4, XSM4, bc,
                                            op=AluOpType.is_equal)
                    nc.vector.tensor_tensor(O4, O4, iot_bc,
                                            op=AluOpType.mult)
                    nc.vector.tensor_tensor(O4[:, :, 0:14, :],
                                            O4[:, :, 0:14, :],
                                            O4[:, :, 14:28, :], op=RMAX)
                    for (lo_, hi_, k_) in ((0, 7, 7), (0, 4, 3), (0, 2, 2),
                                           (0, 1, 1)):
                        nc.vector.tensor_tensor(O4[:, :, lo_:lo_ + k_, :],
                                                O4[:, :, lo_:lo_ + k_, :],
                                                O4[:, :, hi_:hi_ + k_, :],
                                                op=RMAX)
                    nc.scalar.copy(blk2(fa), O4[:, :, 0, :])

                if debug_features:
                    nc.sync.dma_start(y_out[P * it:P * (it + 1), :], F2[:])
                    continue

                # ---- MLP on two N=512 halves (groups 4h..4h+3)
                F2f = F2.rearrange("p (f x) -> p f x", x=GH)
                Y = ypool.tile([P, G * 10], F32, name="yt", tag="yt")
                for hh in (0, 1):
                    fT = [ftpool.tile([nf * H, 512], F32, name=f"ft_{ci}",
                                      tag=f"ft_{ci}")
                          for ci, (f0, nf) in enumerate(TCH)]
                    for gg in range(4):
                        g = 4 * hh + gg
                        fg = fgpool.tile([P, NFEAT], F32, name=f"fg_{gg}",
                                         tag="fg")
                        nc.scalar.copy(
                            fg.rearrange("p (f c) -> p f c", c=H),
                            F2f[:, :, g * H:(g + 1) * H])
                        for ci, (f0, nf) in enumerate(TCH):
                            kc = nf * H
                            pt = psT.tile([kc, P], F32, name=f"pt_{ci}_{gg}",
                                          tag="pt")
                            nc.tensor.transpose(
                                pt[:], fg[:, f0 * H:f0 * H + kc], idn_t[:])
                            nc.scalar.copy(fT[ci][:, gg * P:(gg + 1) * P],
                                           pt[:])

                    a1 = []
                    for mi, (m0, mc) in enumerate(M1):
                        ps = ps1p.tile([mc, 512], F32, name=f"l1_{mi}",
                                       tag="l1")
                        for ci, (k0, kc) in enumerate(K1):
                            nc.tensor.matmul(ps[:],
                                             w1_t[ci][:, m0:m0 + mc],
                                             fT[ci][:],
                                             start=(ci == 0), stop=(ci == 3))
                        am = mpool.tile([mc, 512], F32, name=f"a1_{mi}",
                                        tag=f"a1_{mi}")
                        nc.scalar.activation(am[:], ps[:], RELU,
                                             bias=b1_t[mi][:], scale=1.0)
                        a1.append(am)

                    ps2 = psBp.tile([90, 512], F32, name="l2", tag="lB")
                    for ci, (k0, kc) in enumerate(K2):
                        nc.tensor.matmul(ps2[:], w2_t[ci][0:kc, :],
                                         a1[ci][0:kc, :],
                                         start=(ci == 0), stop=(ci == 2))
                    a2 = mpool.tile([90, 512], F32, name="a2", tag="a2")
                    nc.scalar.activation(a2[:], ps2[:], RELU,
                                         bias=b2_t[:], scale=1.0)

                    ps3 = psBp.tile([30, 512], F32, name="l3", tag="lB")
                    nc.tensor.matmul(ps3[:], w3_t[:], a2[:],
                                     start=True, stop=True)
                    a3 = mpool.tile([30, 512], F32, name="a3", tag="a3")
                    nc.scalar.activation(a3[:], ps3[:], RELU,
                                         bias=b3_t[:], scale=1.0)

                    ps4 = psBp.tile([10, 512], F32, name="l4", tag="lB")
                    nc.tensor.matmul(ps4[:], w4_t[:], a3[:],
                                     start=True, stop=True)
                    ex = mpool.tile([10, 512], F32, name="ex", tag="ex")
                    nc.scalar.activation(ex[:], ps4[:], EXP,
                                         bias=b4_t[:], scale=1.0)

                    for gg in range(4):
                        g = 4 * hh + gg
                        pst = psSp.tile([P, 16], F32, name=f"sm_{gg}",
                                        tag="sm")
                        nc.tensor.transpose(pst[:, 0:10],
                                            ex[:, gg * P:(gg + 1) * P],
                                            idn_t[0:10, 0:10])
                        sumv = mpool.tile([P, 1], F32, name=f"sv_{gg}",
                                          tag="sv")
                        nc.vector.tensor_reduce(sumv[:], pst[:, 0:10],
                                                axis=AXX, op=RADD)
                        rcp = mpool.tile([P, 1], F32, name=f"rc_{gg}",
                                         tag="rc")
                        nc.vector.reciprocal(rcp[:], sumv[:])
                        nc.vector.tensor_scalar_mul(
                            Y[:, g * 10:(g + 1) * 10], pst[:, 0:10], rcp[:])

                nc.sync.dma_start(
                    y_out[it * P * G:(it + 1) * P * G, :]
                        .rearrange("(g p) c -> p g c", p=P),
                    Y.rearrange("p (g c) -> p g c", c=10))

    _split_excess_waits(nc)
    return nc


MAX_WAITS = 1


def _split_excess_waits(nc):
    """Walrus in this container rejects instructions with >MAX_WAITS sem
    waits; hoist the excess onto NoOp carriers inserted just before."""
    import bass_rust
    ctr = [0]
    for f in nc.m.functions:
        for blkk in f.blocks:
            insts = list(blkk.instructions)
            out = []
            changed = False
            for inst in insts:
                si = inst.sync_info
                waits = list(si.on_wait) if (si and si.on_wait) else []
                if len(waits) > MAX_WAITS:
                    changed = True
                    excess = waits[:-MAX_WAITS]
                    si.on_wait = waits[-MAX_WAITS:]
                    for k in range(0, len(excess), MAX_WAITS):
                        nop = bass_rust.InstNoOp(
                            name=f"WSPLIT-{ctr[0]}", ins=[], outs=[])
                        ctr[0] += 1
                        nop.engine = inst.engine
                        nop.sync_info = mybir.SyncInfo(
                            on_wait=excess[k:k + MAX_WAITS], on_update=[])
                        out.append(nop)
                out.append(inst)
            if changed:
                blkk.instructions = out


# ------------------------------------------------------------- numpy driver
def _prep_weights(W1, b1, W2, b2, W3, b3, W4, b4):
    """Fold per-feature affine corrections into W1/b1 (v3 feature order);
    return device weight dict."""
    scale = np.ones(NFEAT, np.float64)
    offset = np.zeros(NFEAT, np.float64)
    ref_off = {n: i * H for i, n in enumerate(REF_ORDER)}
    perm = np.zeros(NFEAT, np.int64)
    for f, name in enumerate(FEATS):
        cols = slice(f * H, (f + 1) * H)
        perm[cols] = ref_off[name] + np.arange(H)
        if name.startswith("mean"):
            scale[cols] = 1.0 / H
        elif name[4] == "i":
            # raw = 1024 - slot  =>  idx = 1024 - raw
            scale[cols] = -1.0
            offset[cols] = IDX_BASE
    W1p = W1.astype(np.float64)[:, perm]
    W1_eff = W1p * scale[None, :]
    b1_eff = b1.astype(np.float64) + W1p @ offset
    iot_row = np.repeat(IDX_BASE - np.arange(H, dtype=np.float32), GH)
    return {
        "w1": np.ascontiguousarray(W1_eff.T.astype(np.float32)),
        "b1": b1_eff.astype(np.float32).reshape(-1, 1),
        "w2": np.ascontiguousarray(W2.T.astype(np.float32)),
        "b2": b2.reshape(-1, 1).astype(np.float32),
        "w3": np.ascontiguousarray(W3.T.astype(np.float32)),
        "b3": b3.reshape(-1, 1).astype(np.float32),
        "w4": np.ascontiguousarray(W4.T.astype(np.float32)),
        "b4": b4.reshape(-1, 1).astype(np.float32),
        "idn": np.eye(P, dtype=np.float32),
        "iot": np.broadcast_to(iot_row[None, :].astype(np.float16),
                               (P, H * GH)).copy(),
    }


_NC_CACHE = {}


def _get_nc(n_st, debug_features=False):
    key = (n_st, debug_features)
    if key not in _NC_CACHE:
        _NC_CACHE[key] = build_nc(n_st, debug_features)
    return _NC_CACHE[key]


def run(t, weights, n_st=N_ST, debug_features=False, trace=False):
    nc = _get_nc(n_st, debug_features)
    rows = P * G * n_st
    in_maps = []
    for c in range(N_CORES):
        m = {"t": np.ascontiguousarray(t[c * B_CORE:c * B_CORE + rows])}
        m.update(weights)
        in_maps.append(m)
    res = run_bass_kernel_spmd(nc, in_maps, core_ids=list(range(N_CORES)),
                               trace=trace)
    outs = [r["y"] for r in res.results]
    return outs, res


def kernel(t, W1, b1, W2, b2, W3, b3, W4, b4):
    weights = _prep_weights(W1, b1, W2, b2, W3, b3, W4, b4)
    outs, _ = run(t, weights)
    y = np.concatenate(outs, axis=0)
    return np.ascontiguousarray(y.astype(np.float32))


# revision 26
# speedup vs baseline: 1.2046x; 1.0080x over previous
"""Trainium2 Bass kernel for nn_CNNModel_42064909697048 (v3).

Per-image row/col stats (min/argmin/max/argmax/mean/median/argmedian over
both axes of each 28x28 image) -> 392 features -> 4-layer MLP -> softmax,
data-parallel over 8 NeuronCores.

v3 design:
- G=8 images per partition per super-tile (1024 images/ST, 16 STs/core).
- Stats computed on an fp16 slot-major copy of each image batch
  ([partition, slot s, 224 groups], slot rows contiguous) so every DVE
  tensor_tensor / scalar_tensor_tensor runs in the 2x packed perf mode.
  Means stay f32 (read the raw input). fp16 stats change the output by
  rel ~6.5e-3 on the actual data (threshold 2e-2).
- Median/min/max via a Batcher odd-even-mergesort network pruned to
  outputs {0,13,27} with per-comparator dead-output elision; min/copy ops
  skipped where the low output is dead.
- argmin/argmax/argmedian: eq-mask vs the extracted value, multiply by a
  static (1024 - slot) tile, per-group reduce-max => first-match index;
  the affine correction (idx = 1024 - raw) is folded into W1/b1 on host.
- MLP restructured to N=512 moving-dim matmuls over staged transposes.

Self-contained: hardcodes shapes/sharding; no sibling imports.
"""

import numpy as np

import concourse.bass as bass
import concourse.mybir as mybir
import concourse.tile as tile_mod
from concourse.tile import TileContext
from concourse.bass_utils import run_bass_kernel_spmd
from concourse.alu_op_type import AluOpType

# ---------------------------------------------------------------- constants
B_TOTAL = 131072
N_CORES = 8
B_CORE = B_TOTAL // N_CORES          # 16384
H = 28
D = 784
P = 128
G = 8                                # images per partition per super-tile
GH = G * H                           # 224 groups per axis view
FD = G * D                           # 6272 elems per partition per tile
N_ST = B_CORE // (P * G)             # 16 super-tiles
MED_IDX = 13
IDX_BASE = 1024.0                    # arg encoding: raw = 1024 - slot
F32 = mybir.dt.float32
F16 = mybir.dt.float16

# v3 feature order (f index 0..13); arg features last 6
FEATS = ["min_v1", "min_v2", "max_v1", "max_v2", "mean_1", "mean_2",
         "med_v1", "med_v2",
         "min_i1", "min_i2", "max_i1", "max_i2", "med_i1", "med_i2"]
REF_ORDER = ["min_v1", "min_i1", "min_v2", "min_i2",
             "max_v1", "max_i1", "max_v2", "max_i2",
             "mean_1", "mean_2",
             "med_v1", "med_i1", "med_v2", "med_i2"]
NFEAT = 392

# Batcher odd-even-mergesort on 28 lines pruned to outputs {0, 13, 27},
# with per-run liveness flags. Entry: (start, dist, step, count,
# need_min_out, need_max_out). After the network slot 0 = min,
# slot 13 = rank-13 (lower median), slot 27 = max.
# UNTOUCHED[r]: slot runs (start, step, count) not written by round r
# (r < 10), copied across the ping-pong buffers by the scalar engine.
UNTOUCHED = [[], [], [(24, 3, 2)], [(3, 1, 2), (11, 1, 2), (19, 1, 2), (23, 4, 2)], [(1, 5, 2), (7, 1, 3), (14, 1, 2), (17, 5, 2), (23, 1, 5)], [(0, 7, 2), (8, 7, 2), (16, 7, 2), (24, 1, 4)], [(0, 7, 2), (8, 7, 2), (16, 5, 2), (22, 1, 2)], [(0, 1, 4), (12, 1, 6), (19, 5, 2)], [(0, 1, 2), (14, 1, 3), (20, 6, 2), (27, 1, 1)], [(0, 15, 2), (16, 1, 3), (27, 1, 1)]]
NET_RUNS = [[(0, 1, 2, 14, True, True)], [(0, 2, 4, 7, True, True), (1, 2, 4, 7, True, True)], [(1, 1, 4, 7, True, True), (0, 4, 8, 3, True, True), (3, 4, 8, 3, True, True)], [(25, 1, 1, 1, True, True), (1, 4, 8, 3, True, True), (2, 4, 8, 3, True, True), (0, 8, 7, 2, True, True), (16, 8, 1, 1, True, True)], [(2, 2, 8, 3, True, True), (3, 2, 8, 3, True, True), (0, 16, 1, 1, True, True)], [(1, 1, 4, 6, True, True), (3, 1, 8, 3, True, True)], [(20, 4, 1, 1, True, True), (1, 8, 1, 6, True, True), (17, 8, 1, 3, True, True)], [(18, 2, 1, 1, True, True), (4, 4, 1, 4, True, True), (21, 4, 1, 3, True, True)], [(17, 1, 1, 1, True, True), (2, 2, 4, 3, True, True), (3, 2, 4, 3, True, True), (19, 2, 3, 2, True, True), (23, 2, 1, 1, True, True)], [(1, 1, 2, 7, True, True), (19, 1, 2, 4, True, True)], [(1, 16, 1, 6, False, True), (8, 16, 1, 3, True, False), (7, 16, 4, 2, True, True)], [(8, 8, 1, 3, False, True), (15, 8, 1, 1, False, True), (11, 8, 1, 4, True, False)], [(7, 4, 5, 2, False, True), (23, 4, 1, 1, False, True), (13, 4, 1, 2, True, False)], [(11, 2, 1, 1, False, True), (14, 2, 1, 1, True, False)], [(13, 1, 1, 1, True, False)]]

# ------------------------------------------------- tile tail-drain workaround
def _patched_drain_and_barrier(self, tick_clock, wait_clock):
    drain_inst = self.nc.sync.drain()
    wait_clock.add_sem_waits(
        drain_inst.ins, tile_mod.ScopedClock({None: tick_clock.global_clock})
    )
    si = drain_inst.ins.sync_info
    waits = list(si.on_wait or [])
    if len(waits) > 1:
        si.on_wait = waits[:1]
        for w in waits[1:]:
            d2 = self.nc.sync.drain()
            si2 = d2.ins.sync_info
            if si2 is None:
                d2.ins.sync_info = mybir.SyncInfo(on_wait=[w], on_update=[])
            else:
                si2.on_wait = [w]
    self.nc.all_engine_barrier()
    assert self.sems is not None
    popped = self.nc._tile_sem_poison_stack.pop()
    assert popped is self._sem_poison
    self.nc.clear_and_free_semaphores(list(self.sems.allocated().values()))
    self.nc.all_engine_barrier()


tile_mod.TileContext._drain_and_barrier = _patched_drain_and_barrier


# ------------------------------------------------------------- bass program
def build_nc(n_st: int = N_ST, debug_features: bool = False):
    nc = bass.Bass()
    rows = P * G * n_st
    t_in = nc.dram_tensor("t", [rows, D], F32, kind="ExternalInput")
    w1 = nc.dram_tensor("w1", [NFEAT, 270], F32, kind="ExternalInput")
    b1 = nc.dram_tensor("b1", [270, 1], F32, kind="ExternalInput")
    w2 = nc.dram_tensor("w2", [270, 90], F32, kind="ExternalInput")
    b2 = nc.dram_tensor("b2", [90, 1], F32, kind="ExternalInput")
    w3 = nc.dram_tensor("w3", [90, 30], F32, kind="ExternalInput")
    b3 = nc.dram_tensor("b3", [30, 1], F32, kind="ExternalInput")
    w4 = nc.dram_tensor("w4", [30, 10], F32, kind="ExternalInput")
    b4 = nc.dram_tensor("b4", [10, 1], F32, kind="ExternalInput")
    idn = nc.dram_tensor("idn", [P, P], F32, kind="ExternalInput")
    iot = nc.dram_tensor("iot", [P, H * GH], F16, kind="ExternalInput")
    if debug_features:
        y_out = nc.dram_tensor("y", [P * n_st, 14 * GH], F32,
                               kind="ExternalOutput")
    else:
        y_out = nc.dram_tensor("y", [rows, 10], F32, kind="ExternalOutput")

    RMAX = AluOpType.max
    RADD = AluOpType.add
    AXX = mybir.AxisListType.X
    RELU = mybir.ActivationFunctionType.Relu
    EXP = mybir.ActivationFunctionType.Exp

    K1 = [(0, 112), (112, 112), (224, 112), (336, 56)]
    M1 = [(0, 128), (128, 128), (256, 14)]
    K2 = [(0, 128), (128, 128), (256, 14)]
    TCH = [(0, 4), (4, 4), (8, 4), (12, 2)]

    with TileContext(nc) as tc:
        with (
            tc.tile_pool(name="wpool", bufs=1) as wpool,
            tc.tile_pool(name="xpool", bufs=2) as xpool,
            tc.tile_pool(name="smpool", bufs=1) as smpool,
            tc.tile_pool(name="vpool", bufs=1) as vpool,
            tc.tile_pool(name="opool", bufs=1) as opool,
            tc.tile_pool(name="fpool", bufs=2) as fpool,
            tc.tile_pool(name="ftpool", bufs=1) as ftpool,
            tc.tile_pool(name="fgpool", bufs=2) as fgpool,
            tc.tile_pool(name="mpool", bufs=1) as mpool,
            tc.tile_pool(name="ypool", bufs=2) as ypool,
            tc.tile_pool(name="psT", bufs=2, space="PSUM") as psT,
            tc.tile_pool(name="ps1", bufs=2, space="PSUM") as ps1p,
            tc.tile_pool(name="psB", bufs=2, space="PSUM") as psBp,
            tc.tile_pool(name="psS", bufs=2, space="PSUM") as psSp,
        ):
            # ---- first input tile DMA ahead of the weight DMAs (weights are
            # first used late in ST0, so this unblocks ST0's stats sooner)
            X0 = xpool.tile([P, FD], F32, name="x", tag="x")
            nc.sync.dma_start(
                X0.rearrange("p (g d) -> p g d", d=D),
                t_in[0:P * G, :].rearrange("(g p) d -> p g d", p=P))

            # ---- static weights into SBUF
            w1_t = []
            for ci, (k0, kc) in enumerate(K1):
                wt = wpool.tile([kc, 270], F32, name=f"w1_{ci}", tag=f"w1_{ci}")
                nc.sync.dma_start(wt[:], w1[k0:k0 + kc, :])
                w1_t.append(wt)
            b1_t = []
            for mi, (m0, mc) in enumerate(M1):
                bt = wpool.tile([mc, 1], F32, name=f"b1_{mi}", tag=f"b1_{mi}")
                nc.sync.dma_start(bt[:], b1[m0:m0 + mc, :])
                b1_t.append(bt)
            w2_t = []
            for ci, (k0, kc) in enumerate(K2):
                wt = wpool.tile([kc, 90], F32, name=f"w2_{ci}", tag=f"w2_{ci}")
                nc.sync.dma_start(wt[:], w2[k0:k0 + kc, :])
                w2_t.append(wt)
            b2_t = wpool.tile([90, 1], F32, name="b2", tag="b2")
            nc.sync.dma_start(b2_t[:], b2[:, :])
            w3_t = wpool.tile([90, 30], F32, name="w3", tag="w3")
            nc.sync.dma_start(w3_t[:], w3[:, :])
            b3_t = wpool.tile([30, 1], F32, name="b3", tag="b3")
            nc.sync.dma_start(b3_t[:], b3[:, :])
            w4_t = wpool.tile([30, 10], F32, name="w4", tag="w4")
            nc.sync.dma_start(w4_t[:], w4[:, :])
            b4_t = wpool.tile([10, 1], F32, name="b4", tag="b4")
            nc.sync.dma_start(b4_t[:], b4[:, :])
            idn_t = wpool.tile([P, P], F32, name="idn", tag="idn")
            nc.sync.dma_start(idn_t[:], idn[:, :])
            iot_t = wpool.tile([P, H * GH], F16, name="iot", tag="iot")
            nc.sync.dma_start(iot_t[:], iot[:, :])

            for it in range(n_st):
                if it == 0:
                    X = X0
                else:
                    X = xpool.tile([P, FD], F32, name="x", tag="x")
                    nc.sync.dma_start(
                        X.rearrange("p (g d) -> p g d", d=D),
                        t_in[it * P * G:(it + 1) * P * G, :]
                            .rearrange("(g p) d -> p g d", p=P))

                # dual-axis slot-major fp16 copy XSM[p, a, s, x]:
                #   a=0 (axis1): XSM[p,0,r,(g c)] ; a=1 (axis2): XSM[p,1,c,(g r)]
                XSM = smpool.tile([P, 2 * H * GH], F16, name="xsm", tag="xsm")
                XSM4 = XSM.rearrange("p (a s x) -> p a s x", s=H, x=GH)
                nc.scalar.copy(
                    XSM[:, 0:H * GH].rearrange("p (r g c) -> p r g c",
                                               g=G, c=H),
                    X.rearrange("p (g r c) -> p r g c", r=H, c=H))
                nc.scalar.copy(
                    XSM[:, H * GH:].rearrange("p (s x) -> p s x", x=GH),
                    X.rearrange("p (x c) -> p c x", c=H))

                F2 = fpool.tile([P, 14 * GH], F32, name="feat", tag="feat")
                TGT = fpool.tile([P, 8 * GH], F16, name="tgt", tag="tgt")

                def blk2(f):
                    return F2[:, f * GH:(f + 2) * GH] \
                        .rearrange("p (a x) -> p a x", x=GH)

                # ---- means: fp16 fold-tree over slots (sum), staged in O
                # (free until sort round 0 overwrites it), both axes at once
                O = opool.tile([P, 2 * H * GH], F16, name="osel", tag="osel")
                O4 = O.rearrange("p (a s x) -> p a s x", s=H, x=GH)
                nc.vector.tensor_tensor(O4[:, :, 0:14, :],
                                        XSM4[:, :, 0:14, :],
                                        XSM4[:, :, 14:28, :], op=RADD)
                for (lo_, hi_, k_) in ((0, 7, 7), (0, 4, 3), (0, 2, 2),
                                       (0, 1, 1)):
                    nc.vector.tensor_tensor(O4[:, :, lo_:lo_ + k_, :],
                                            O4[:, :, lo_:lo_ + k_, :],
                                            O4[:, :, hi_:hi_ + k_, :],
                                            op=RADD)
                nc.scalar.copy(blk2(4), O4[:, :, 0, :])

                # ---- dual-axis sort on fp16 slot-major scratch.
                # Rounds 0-9 ping-pong XSM -> O -> V -> O -> ... (min/max write
                # the other buffer: no temp, no copy-back; scalar copies only
                # the untouched slots, concurrently). Rounds 10+ run in-place
                # on V with liveness flags.
                V = vpool.tile([P, 2 * H * GH], F16, name="vsort", tag="vsort")
                V4 = V.rearrange("p (a s x) -> p a s x", s=H, x=GH)

                def _sl(st_, step, n):
                    return slice(st_, st_ + step * (n - 1) + 1, step) \
                        if n > 1 else slice(st_, st_ + 1)

                for r, runs in enumerate(NET_RUNS):
                    if r < 10:
                        bin_ = XSM4 if r == 0 else (O4 if r % 2 == 1 else V4)
                        bout = O4 if r % 2 == 0 else V4
                        for (st_, d, step, n, nm, nx) in runs:
                            sl = _sl(st_, step, n)
                            sh = _sl(st_ + d, step, n)
                            nc.vector.tensor_tensor(
                                bout[:, :, sl, :], bin_[:, :, sl, :],
                                bin_[:, :, sh, :], op=AluOpType.min)
                            nc.vector.tensor_tensor(
                                bout[:, :, sh, :], bin_[:, :, sl, :],
                                bin_[:, :, sh, :], op=RMAX)
                        for (u0, ustep, ucnt) in UNTOUCHED[r]:
                            us = _sl(u0, ustep, ucnt)
                            nc.scalar.copy(bout[:, :, us, :],
                                           bin_[:, :, us, :])
                    else:
                        for (st_, d, step, n, nm, nx) in runs:
                            sl = _sl(st_, step, n)
                            sh = _sl(st_ + d, step, n)
                            lo = V4[:, :, sl, :]
                            hi = V4[:, :, sh, :]
                            if nm and nx:
                                tt = O4[:, :, 0:n, :]
                                nc.vector.tensor_tensor(tt, lo, hi,
                                                        op=AluOpType.min)
                                nc.vector.tensor_tensor(hi, lo, hi, op=RMAX)
                                nc.scalar.copy(lo, tt)
                            elif nm:
                                nc.vector.tensor_tensor(lo, lo, hi,
                                                        op=AluOpType.min)
                            else:
                                nc.vector.tensor_tensor(hi, lo, hi, op=RMAX)
                # extract value features (axis pairs adjacent in F2/TGT):
                # slot0 -> (min_v1,min_v2)=f0,1; slot13 -> f6,7; slot27 -> f2,3
                for s_, f_ in ((0, 0), (27, 2), (MED_IDX, 6)):
                    nc.scalar.copy(blk2(f_), V4[:, :, s_, :])
                    nc.scalar.copy(
                        TGT[:, f_ * GH:(f_ + 2) * GH]
                            .rearrange("p (a x) -> p a x", x=GH),
                        V4[:, :, s_, :])

                # ---- arg features, axis-paired: eq-mask * (1024 - slot),
                # fp16 fold-tree max => first-match index (O reused as scratch)
                iot_bc = iot_t.rearrange("p (u s x) -> p u s x", u=1, x=GH) \
                              .broadcast_to([P, 2, H, GH])
                for (fa, fv) in ((8, 0), (10, 2), (12, 6)):
                    bc = TGT[:, fv * GH:(fv + 2) * GH] \
                        .rearrange("p (a u x) -> p a u x", u=1, x=GH) \
                        .broadcast_to([P, 2, H, GH])
                    nc.vector.tensor_tensor(O4, XSM4, bc,
                                            op=AluOpType.is_equal)
                    nc.vector.tensor_tensor(O4, O4, iot_bc,
                                            op=AluOpType.mult)
                    nc.vector.tensor_tensor(O4[:, :, 0:14, :],
                                            O4[:, :, 0:14, :],
                                            O4[:, :, 14:28, :], op=RMAX)
                    for (lo_, hi_, k_) in ((0, 7, 7), (0, 4, 3), (0, 2, 2),
                                           (0, 1, 1)):
                        nc.vector.tensor_tensor(O4[:, :, lo_:lo_ + k_, :],
                                                O4[:, :, lo_:lo_ + k_, :],
                                                O4[:, :, hi_:hi_ + k_, :],
                                                op=RMAX)
                    nc.scalar.copy(blk2(fa), O4[:, :, 0, :])

                if debug_features:
                    nc.sync.dma_start(y_out[P * it:P * (it + 1), :], F2[:])
                    continue

                # ---- MLP on two N=512 halves (groups 4h..4h+3)
                F2f = F2.rearrange("p (f x) -> p f x", x=GH)
                Y = ypool.tile([P, G * 10], F32, name="yt", tag="yt")
                for hh in (0, 1):
                    fT = [ftpool.tile([nf * H, 512], F32, name=f"ft_{ci}",
                                      tag=f"ft_{ci}")
                          for ci, (f0, nf) in enumerate(TCH)]
                    for gg in range(4):
                        g = 4 * hh + gg
                        fg = fgpool.tile([P, NFEAT], F32, name=f"fg_{gg}",
                                         tag="fg")
                        nc.scalar.copy(
                            fg.rearrange("p (f c) -> p f c", c=H),
                            F2f[:, :, g * H:(g + 1) * H])
                        for ci, (f0, nf) in enumerate(TCH):
                            kc = nf * H
                            pt = psT.tile([kc, P], F32, name=f"pt_{ci}_{gg}",
                                          tag="pt")
                            nc.tensor.transpose(
                                pt[:], fg[:, f0 * H:f0 * H + kc], idn_t[:])
                            nc.scalar.copy(fT[ci][:, gg * P:(gg + 1) * P],
                                           pt[:])

                    a1 = []
                    for mi, (m0, mc) in enumerate(M1):
                        ps = ps1p.tile([mc, 512], F32, name=f"l1_{mi}",
                                       tag="l1")
                        for ci, (k0, kc) in enumerate(K1):
                            nc.tensor.matmul(ps[:],
                                             w1_t[ci][:, m0:m0 + mc],
                                             fT[ci][:],
                                             start=(ci == 0), stop=(ci == 3))
                        am = mpool.tile([mc, 512], F32, name=f"a1_{mi}",
                                        tag=f"a1_{mi}")
                        nc.scalar.activation(am[:], ps[:], RELU,
                                             bias=b1_t[mi][:], scale=1.0)
                        a1.append(am)

                    ps2 = psBp.tile([90, 512], F32, name="l2", tag="lB")
                    for ci, (k0, kc) in enumerate(K2):
                        nc.tensor.matmul(ps2[:], w2_t[ci][0:kc, :],
                                         a1[ci][0:kc, :],
                                         start=(ci == 0), stop=(ci == 2))
                    a2 = mpool.tile([90, 512], F32, name="a2", tag="a2")
                    nc.scalar.activation(a2[:], ps2[:], RELU,
                                         bias=b2_t[:], scale=1.0)

                    ps3 = psBp.tile([30, 512], F32, name="l3", tag="lB")
                    nc.tensor.matmul(ps3[:], w3_t[:], a2[:],
                                     start=True, stop=True)
                    a3 = mpool.tile([30, 512], F32, name="a3", tag="a3")
                    nc.scalar.activation(a3[:], ps3[:], RELU,
                                         bias=b3_t[:], scale=1.0)

                    ps4 = psBp.tile([10, 512], F32, name="l4", tag="lB")
                    nc.tensor.matmul(ps4[:], w4_t[:], a3[:],
                                     start=True, stop=True)
                    ex = mpool.tile([10, 512], F32, name="ex", tag="ex")
                    nc.scalar.activation(ex[:], ps4[:], EXP,
                                         bias=b4_t[:], scale=1.0)

                    for gg in range(4):
                        g = 4 * hh + gg
                        pst = psSp.tile([P, 16], F32, name=f"sm_{gg}",
                                        tag="sm")
                        nc.tensor.transpose(pst[:, 0:10],
                                            ex[:, gg * P:(gg + 1) * P],
                                            idn_t[0:10, 0:10])
                        sumv = mpool.tile([P, 1], F32, name=f"sv_{gg}",
                                          tag="sv")
                        nc.vector.tensor_reduce(sumv[:], pst[:, 0:10],
                                                axis=AXX, op=RADD)
                        rcp = mpool.tile([P, 1], F32, name=f"rc_{gg}",
                                         tag="rc")
                        nc.vector.reciprocal(rcp[:], sumv[:])
                        nc.vector.tensor_scalar_mul(
                            Y[:, g * 10:(g + 1) * 10], pst[:, 0:10], rcp[:])

                nc.sync.dma_start(
                    y_out[it * P * G:(it + 1) * P * G, :]
                        .rearrange("(g p) c -> p g c", p=P),
                    Y.rearrange("p (g c) -> p g c", c=10))

    _split_excess_waits(nc)
    return nc


MAX_WAITS = 1


def _split_excess_waits(nc):
    """Walrus in this container rejects instructions with >MAX_WAITS sem
    waits; hoist the excess onto NoOp carriers inserted just before."""
    import bass_rust
    ctr = [0]
    for f in nc.m.functions:
        for blkk in f.blocks:
            insts = list(blkk.instructions)
            out = []
            changed = False
            for inst in insts:
                si = inst.sync_info
                waits = list(si.on_wait) if (si and si.on_wait) else []
                if len(waits) > MAX_WAITS:
                    changed = True
                    excess = waits[:-MAX_WAITS]
                    si.on_wait = waits[-MAX_WAITS:]
                    for k in range(0, len(excess), MAX_WAITS):
                        nop = bass_rust.InstNoOp(
                            name=f"WSPLIT-{ctr[0]}", ins=[], outs=[])
                        ctr[0] += 1
                        nop.engine = inst.engine
                        nop.sync_info = mybir.SyncInfo(
                            on_wait=excess[k:k + MAX_WAITS], on_update=[])
                        out.append(nop)
                out.append(inst)
            if changed:
                blkk.instructions = out


# ------------------------------------------------------------- numpy driver
def _prep_weights(W1, b1, W2, b2, W3, b3, W4, b4):
    """Fold per-feature affine corrections into W1/b1 (v3 feature order);
    return device weight dict."""
    scale = np.ones(NFEAT, np.float64)
    offset = np.zeros(NFEAT, np.float64)
    ref_off = {n: i * H for i, n in enumerate(REF_ORDER)}
    perm = np.zeros(NFEAT, np.int64)
    for f, name in enumerate(FEATS):
        cols = slice(f * H, (f + 1) * H)
        perm[cols] = ref_off[name] + np.arange(H)
        if name.startswith("mean"):
            scale[cols] = 1.0 / H
        elif name[4] == "i":
            # raw = 1024 - slot  =>  idx = 1024 - raw
            scale[cols] = -1.0
            offset[cols] = IDX_BASE
    W1p = W1.astype(np.float64)[:, perm]
    W1_eff = W1p * scale[None, :]
    b1_eff = b1.astype(np.float64) + W1p @ offset
    iot_row = np.repeat(IDX_BASE - np.arange(H, dtype=np.float32), GH)
    return {
        "w1": np.ascontiguousarray(W1_eff.T.astype(np.float32)),
        "b1": b1_eff.astype(np.float32).reshape(-1, 1),
        "w2": np.ascontiguousarray(W2.T.astype(np.float32)),
        "b2": b2.reshape(-1, 1).astype(np.float32),
        "w3": np.ascontiguousarray(W3.T.astype(np.float32)),
        "b3": b3.reshape(-1, 1).astype(np.float32),
        "w4": np.ascontiguousarray(W4.T.astype(np.float32)),
        "b4": b4.reshape(-1, 1).astype(np.float32),
        "idn": np.eye(P, dtype=np.float32),
        "iot": np.broadcast_to(iot_row[None, :].astype(np.float16),
                               (P, H * GH)).copy(),
    }


_NC_CACHE = {}


def _get_nc(n_st, debug_features=False):
    key = (n_st, debug_features)
    if key not in _NC_CACHE:
        _NC_CACHE[key] = build_nc(n_st, debug_features)
    return _NC_CACHE[key]


def run(t, weights, n_st=N_ST, debug_features=False, trace=False):
    nc = _get_nc(n_st, debug_features)
    rows = P * G * n_st
    in_maps = []
    for c in range(N_CORES):
        m = {"t": np.ascontiguousarray(t[c * B_CORE:c * B_CORE + rows])}
        m.update(weights)
        in_maps.append(m)
    res = run_bass_kernel_spmd(nc, in_maps, core_ids=list(range(N_CORES)),
                               trace=trace)
    outs = [r["y"] for r in res.results]
    return outs, res


def kernel(t, W1, b1, W2, b2, W3, b3, W4, b4):
    weights = _prep_weights(W1, b1, W2, b2, W3, b3, W4, b4)
    outs, _ = run(t, weights)
    y = np.concatenate(outs, axis=0)
    return np.ascontiguousarray(y.astype(np.float32))
